# revision 10
# baseline (speedup 1.0000x reference)
"""DeltaDequantization Trainium2 kernel (8-core SPMD, pure data parallel over batch).

Math (per batch element b, chunks c of 32 steps):
    scale_c = (1/32) * sum_{s,n} x[b,c,s,n] * cs[n]          (independent of carry!)
    S_c     = prod_{c'<c} scale_c'          (exclusive cumprod)
    y[b,t]  = sum_n x[b,t,n] * qb[n]
    m_c     = (1/32) * sum_{s in c} y[b,t]
    pred_c  = sum_{c'<c} S_c' * m_c'        (exclusive cumsum)
    out[b,t]= pred_c(t) + S_c(t) * y[b,t]

Kernel: load x naturally [128b, (t,n)], PE-transpose 128x128 blocks to get
(t',n) on partitions, one [128,8] matmul computes y and w=x@cs for 4 t's at a
time, PE-transpose back to [b,t], tensor_tensor_scan for the 64-step
recurrences, affine, store.

Pipelining: span sp's post-matmul stages (back-transpose, y/w copies, chunk
reductions, scan segments, affine, store) are EMITTED inside span sp+1's pb
loop at points where their dependencies are already satisfied.  Engine queues
execute in (priority = program) order with head-of-line blocking, so emitting
the post stages too early stalls the next span's PSUM-drain copies (PE backs
up, the SWDGE load stream dips), and emitting them after the next span's loop
pushes them onto the serial tail.
"""

import numpy as np
from contextlib import ExitStack

import concourse.bass as bass
import concourse.bacc as bacc
import concourse.tile as tile
from concourse import mybir
from concourse.bass_utils import run_bass_kernel_spmd
from concourse.masks import make_identity

F32 = mybir.dt.float32
BF16 = mybir.dt.bfloat16

B, T, NB = 1024, 2048, 32
NCORES = 8
BS = B // NCORES          # 128 batch rows per core = full partition dim
ADAPT = 32
C = T // ADAPT            # 64 chunks
SPAN_T = 256              # timesteps per pipelined span
NSPAN = T // SPAN_T       # 8
SPAN_F = SPAN_T * NB      # 8192 f32 elements per partition per span
CPS = SPAN_T // ADAPT     # 8 chunks per span

_cached_nc = None


def build_kernel():
    nc = bacc.Bacc("TRN2", target_bir_lowering=False, debug=False)

    x_ext = nc.dram_tensor("x", [BS, T * NB], F32, kind="ExternalInput")
    qb_ext = nc.dram_tensor("quant_bins", [NB, 1], F32, kind="ExternalInput")
    cs_ext = nc.dram_tensor("change_scales", [NB, 1], F32, kind="ExternalInput")
    out_ext = nc.dram_tensor("out", [BS, T], F32, kind="ExternalOutput")

    with tile.TileContext(nc) as tc:
        with (
            tc.tile_pool(name="consts", bufs=1) as consts,
            tc.tile_pool(name="xpool", bufs=6) as xpool,
            tc.tile_pool(name="xtpool", bufs=3) as xtpool,
            tc.tile_pool(name="midpool", bufs=2) as midpool,
            tc.tile_pool(name="accpool", bufs=1) as accpool,
            tc.tile_pool(name="smallpool", bufs=1) as smallpool,
            tc.tile_pool(name="ps_t", bufs=4, space="PSUM") as ps_t,
            tc.tile_pool(name="ps_yw", bufs=2, space="PSUM") as ps_yw,
            tc.tile_pool(name="ps_b", bufs=2, space="PSUM") as ps_b,
        ):
            # Issue the first x cast-loads before anything else touches the
            # GpSimd queue so HBM streaming starts at the preamble's end.
            # Half-span grain (16 KiB contiguous f32 per partition): the 16
            # SWDGE engines run 100% busy at ~25 B/ns HBM-side with 8 KiB
            # rows (~60 ns/packet overhead); doubling the row halves the
            # packet count per byte.
            HSF = SPAN_F // 2  # half-span: 128 t = 2 MiB of f32 in DRAM
            NL = NSPAN * 2
            PREF = 4           # lookahead in half-span loads (= 2 spans)
            xq = []

            def issue_load(li):
                x_h = xpool.tile([128, HSF], BF16)
                if li == NL - 1:
                    # Final half-span split in four: lets the tail's
                    # transposes start while later pieces are in flight.
                    h = HSF // 4
                    for i in range(4):
                        nc.gpsimd.dma_start(
                            out=x_h[:, i * h:(i + 1) * h],
                            in_=x_ext[:, li * HSF + i * h:li * HSF + (i + 1) * h],
                        )
                else:
                    nc.gpsimd.dma_start(
                        out=x_h[:], in_=x_ext[:, li * HSF:(li + 1) * HSF]
                    )
                xq.append(x_h)

            issue_load(0)
            issue_load(1)

            ident = consts.tile([128, 128], F32)
            make_identity(nc, ident[:])
            ident_bf = consts.tile([128, 128], BF16)
            make_identity(nc, ident_bf[:])

            for li in range(2, PREF):
                issue_load(li)

            # Four stationary matrices A32_q [128, 32], q = 0..3.
            # Column m = 16*j + 4*q + t''; A32_q[(t', n), m] = delta(t', t'') *
            # (qb[n] if j == 0 else cs[n]); zero columns for other q values.
            # Stage qb/cs via the Activation HWDGE queue: the Sync queue
            # carries one companion descriptor per SWDGE load, so qbcs
            # staging there would delay the x stream.
            qbcs = consts.tile([128, 2], F32)
            for tp in range(4):
                nc.scalar.dma_start(out=qbcs[32 * tp:32 * tp + 32, 0:1], in_=qb_ext[:])
                nc.scalar.dma_start(out=qbcs[32 * tp:32 * tp + 32, 1:2], in_=cs_ext[:])
            A32 = []
            for q in range(4):
                Aq = consts.tile([128, 32], BF16, tag=f"A32_{q}")
                nc.vector.memset(Aq[:], 0.0)
                for tp in range(4):
                    sl = slice(32 * tp, 32 * tp + 32)
                    nc.vector.tensor_copy(
                        out=Aq[sl, 4 * q + tp:4 * q + tp + 1], in_=qbcs[sl, 0:1]
                    )
                    nc.vector.tensor_copy(
                        out=Aq[sl, 16 + 4 * q + tp:16 + 4 * q + tp + 1], in_=qbcs[sl, 1:2]
                    )
                A32.append(Aq)

            # Persistent per-core accumulators
            y_sb = accpool.tile([128, T], F32)
            w_sb = accpool.tile([128, T], F32)
            out_sb = accpool.tile([128, T], F32)

            m_term = smallpool.tile([128, C], F32)
            p_sc = smallpool.tile([128, C], F32)
            S_exc = smallpool.tile([128, C + 1], F32)
            pred = smallpool.tile([128, C + 1], F32)
            tau = smallpool.tile([128, C], F32)
            nc.vector.memset(S_exc[:, 0:1], 1.0)
            nc.vector.memset(pred[:, 0:1], 0.0)

            yw_mid_of = {}
            ps2_of = {}

            def emit_ywmid(sp, ps_y):
                yw_mid = midpool.tile([128, 512], F32)
                nc.scalar.copy(out=yw_mid[:], in_=ps_y[:])
                yw_mid_of[sp] = yw_mid

            def emit_backT(sp):
                yw_mid = yw_mid_of.pop(sp)
                ps2 = ps_b.tile([128, 512], F32)
                for blk2 in range(4):
                    nc.tensor.transpose(
                        ps2[:, blk2 * 128:(blk2 + 1) * 128],
                        yw_mid[:, blk2 * 128:(blk2 + 1) * 128],
                        ident[:],
                    )
                ps2_of[sp] = ps2

            def emit_ywcopies_reduce(sp):
                # ps2 free index = 128*blk + 32*g4 + 16*j + 4*q + t''
                # t(within span) = 64*g4 + 16*q + 4*blk + t''
                ps2 = ps2_of.pop(sp)
                ps2v = ps2[:].rearrange(
                    "p (b g j q t) -> p g q b j t", b=4, g=4, j=2, q=4, t=4
                )
                yspan = y_sb[:, sp * SPAN_T:(sp + 1) * SPAN_T].rearrange(
                    "p (g q b t) -> p g q b t", g=4, q=4, b=4, t=4
                )
                wspan = w_sb[:, sp * SPAN_T:(sp + 1) * SPAN_T].rearrange(
                    "p (g q b t) -> p g q b t", g=4, q=4, b=4, t=4
                )
                nc.vector.tensor_copy(out=yspan, in_=ps2v[:, :, :, :, 0:1, :].squeeze(4))
                nc.vector.tensor_copy(out=wspan, in_=ps2v[:, :, :, :, 1:2, :].squeeze(4))

                csl = slice(sp * CPS, (sp + 1) * CPS)
                nc.vector.tensor_reduce(
                    out=m_term[:, csl],
                    in_=y_sb[:, sp * SPAN_T:(sp + 1) * SPAN_T].rearrange(
                        "p (c s) -> p c s", c=CPS, s=ADAPT
                    ),
                    axis=mybir.AxisListType.X,
                    op=mybir.AluOpType.add,
                )
                nc.vector.tensor_reduce(
                    out=p_sc[:, csl],
                    in_=w_sb[:, sp * SPAN_T:(sp + 1) * SPAN_T].rearrange(
                        "p (c s) -> p c s", c=CPS, s=ADAPT
                    ),
                    axis=mybir.AxisListType.X,
                    op=mybir.AluOpType.add,
                )

            def emit_scans(sp):
                c_lo, c_hi = sp * CPS, (sp + 1) * CPS
                sl = slice(c_lo, c_hi)
                nc.vector.tensor_scalar_mul(m_term[:, sl], m_term[:, sl], 1.0 / ADAPT)
                nc.vector.tensor_scalar_mul(p_sc[:, sl], p_sc[:, sl], 1.0 / ADAPT)
                nc.vector.tensor_tensor_scan(
                    out=S_exc[:, c_lo + 1:c_hi + 1],
                    data0=p_sc[:, sl],
                    data1=p_sc[:, sl],
                    initial=S_exc[:, c_lo:c_lo + 1],
                    op0=mybir.AluOpType.mult,
                    op1=mybir.AluOpType.bypass,
                )
                nc.vector.tensor_mul(out=tau[:, sl], in0=S_exc[:, sl], in1=m_term[:, sl])
                nc.vector.tensor_tensor_scan(
                    out=pred[:, c_lo + 1:c_hi + 1],
                    data0=tau[:, sl],
                    data1=tau[:, sl],
                    initial=pred[:, c_lo:c_lo + 1],
                    op0=mybir.AluOpType.add,
                    op1=mybir.AluOpType.bypass,
                )

            def emit_affine_store(sp):
                c_lo, c_hi = sp * CPS, (sp + 1) * CPS
                sl = slice(c_lo, c_hi)
                t_lo, t_hi = c_lo * ADAPT, c_hi * ADAPT
                y3 = y_sb[:, t_lo:t_hi].rearrange("p (c s) -> p c s", c=CPS, s=ADAPT)
                o3 = out_sb[:, t_lo:t_hi].rearrange("p (c s) -> p c s", c=CPS, s=ADAPT)
                S_b = S_exc[:, sl].unsqueeze(2).broadcast_to([128, CPS, ADAPT])
                pred_b = pred[:, sl].unsqueeze(2).broadcast_to([128, CPS, ADAPT])
                nc.vector.tensor_mul(out=o3, in0=y3, in1=S_b)
                nc.vector.tensor_add(out=o3, in0=o3, in1=pred_b)
                nc.sync.dma_start(out=out_ext[:, t_lo:t_hi], in_=out_sb[:, t_lo:t_hi])

            for sp in range(NSPAN):
                xT_sp = xtpool.tile([128, SPAN_F], BF16)
                # y/w projection, interleaved with the transposes that feed it.
                # Group g = g4*4+q covers blocks 4g..4g+3 (t = 16g + 4blk + t'').
                # Strip g4 accumulates 4 matmuls into ps_y[32*g4 : 32*g4+32, :];
                # dense partition layout p = 32*g4 + 16*j + 4*q + t'', free=(blk,b).
                ps_y = ps_yw.tile([128, 512], F32)
                for pb in range(8):
                    # SWDGE cast-load f32 DRAM -> bf16 SBUF, half-span grain
                    if pb % 4 == 0:
                        li = sp * 2 + pb // 4
                        if li + PREF < NL:
                            issue_load(li + PREF)
                        x_h = xq[li]
                    pst = ps_t.tile([128, 1024], BF16)
                    for k in range(8):
                        blk_l = (pb % 4) * 8 + k
                        nc.tensor.transpose(
                            pst[:, k * 128:(k + 1) * 128],
                            x_h[:, blk_l * 128:(blk_l + 1) * 128],
                            ident_bf[:],
                        )
                    dst = xT_sp[:, pb * 1024:(pb + 1) * 1024]
                    if pb % 2 == 0:
                        nc.vector.tensor_copy(out=dst, in_=pst[:])
                    else:
                        nc.scalar.copy(out=dst, in_=pst[:])
                    for gg in range(2):
                        g = pb * 2 + gg
                        g4, q = divmod(g, 4)
                        nc.tensor.matmul(
                            ps_y[32 * g4:32 * g4 + 32, :],
                            A32[q][:],
                            xT_sp[:, g * 512:(g + 1) * 512],
                            start=(q == 0),
                            stop=(q == 3),
                            tile_position=(0, 32 * g4),
                        )
                    # Previous span's post stages, placed where their deps are
                    # already met so they never head-of-line-block this span.
                    if sp > 0:
                        if pb == 1:
                            emit_backT(sp - 1)
                        elif pb == 3:
                            emit_ywcopies_reduce(sp - 1)
                        elif pb == 5:
                            emit_scans(sp - 1)
                        elif pb == 7:
                            emit_affine_store(sp - 1)

                emit_ywmid(sp, ps_y)

            last = NSPAN - 1
            emit_backT(last)
            emit_ywcopies_reduce(last)
            emit_scans(last)
            emit_affine_store(last)

    nc.compile()
    return nc


def kernel(x, quant_bins, change_scales):
    global _cached_nc
    if _cached_nc is None:
        _cached_nc = build_kernel()
    nc = _cached_nc

    x = np.ascontiguousarray(x, dtype=np.float32)
    qb = np.ascontiguousarray(quant_bins, dtype=np.float32).reshape(NB, 1)
    cs = np.ascontiguousarray(change_scales, dtype=np.float32).reshape(NB, 1)

    in_maps = [
        {
            "x": x[i * BS:(i + 1) * BS].reshape(BS, T * NB),
            "quant_bins": qb,
            "change_scales": cs,
        }
        for i in range(NCORES)
    ]
    res = run_bass_kernel_spmd(nc, in_maps, core_ids=list(range(NCORES)))
    out = np.concatenate([res.results[i]["out"] for i in range(NCORES)], axis=0)
    return out.astype(np.float32)


if __name__ == "__main__":
    rng = np.random.default_rng(0)
    x = rng.standard_normal((B, T, NB)).astype(np.float32)
    qb = rng.standard_normal((NB,)).astype(np.float32)
    cs = rng.uniform(0.9, 1.1, (NB, 1)).astype(np.float32)
    out = kernel(x=x, quant_bins=qb, change_scales=cs)
    print("out", out.shape, out.dtype)


# revision 13
# speedup vs baseline: 1.1781x; 1.1781x over previous
"""DeltaDequantization Trainium2 kernel (8-core SPMD, pure data parallel over batch).

Math (per batch element b, chunks c of 32 steps):
    scale_c = (1/32) * sum_{s,n} x[b,c,s,n] * cs[n]          (independent of carry!)
    S_c     = prod_{c'<c} scale_c'          (exclusive cumprod)
    y[b,t]  = sum_n x[b,t,n] * qb[n]
    m_c     = (1/32) * sum_{s in c} y[b,t]
    pred_c  = sum_{c'<c} S_c' * m_c'        (exclusive cumsum)
    out[b,t]= pred_c(t) + S_c(t) * y[b,t]

Kernel: load x naturally [128b, (t,n)], PE-transpose 128x128 blocks to get
(t',n) on partitions, one [128,8] matmul computes y and w=x@cs for 4 t's at a
time, PE-transpose back to [b,t], tensor_tensor_scan for the 64-step
recurrences, affine, store.

Pipelining: span sp's post-matmul stages (back-transpose, y/w copies, chunk
reductions, scan segments, affine, store) are EMITTED inside span sp+1's pb
loop at points where their dependencies are already satisfied.  Engine queues
execute in (priority = program) order with head-of-line blocking, so emitting
the post stages too early stalls the next span's PSUM-drain copies (PE backs
up, the SWDGE load stream dips), and emitting them after the next span's loop
pushes them onto the serial tail.
"""

import numpy as np
from contextlib import ExitStack

import concourse.bass as bass
import concourse.bacc as bacc
import concourse.tile as tile
from concourse import mybir
from concourse.bass_utils import run_bass_kernel_spmd
from concourse.masks import make_identity

F32 = mybir.dt.float32
BF16 = mybir.dt.bfloat16

B, T, NB = 1024, 2048, 32
NCORES = 8
BS = B // NCORES          # 128 batch rows per core = full partition dim
ADAPT = 32
C = T // ADAPT            # 64 chunks
SPAN_T = 256              # timesteps per pipelined span
NSPAN = T // SPAN_T       # 8
SPAN_F = SPAN_T * NB      # 8192 f32 elements per partition per span
CPS = SPAN_T // ADAPT     # 8 chunks per span

_cached_nc = None


def build_kernel():
    nc = bacc.Bacc("TRN2", target_bir_lowering=False, debug=False)

    x_ext = nc.dram_tensor("x", [BS, T * NB], F32, kind="ExternalInput")
    qb_ext = nc.dram_tensor("quant_bins", [NB, 1], F32, kind="ExternalInput")
    cs_ext = nc.dram_tensor("change_scales", [NB, 1], F32, kind="ExternalInput")
    out_ext = nc.dram_tensor("out", [BS, T], F32, kind="ExternalOutput")

    with tile.TileContext(nc) as tc:
        with (
            tc.tile_pool(name="consts", bufs=1) as consts,
            tc.tile_pool(name="xpool", bufs=10) as xpool,
            tc.tile_pool(name="xtpool", bufs=3) as xtpool,
            tc.tile_pool(name="midpool", bufs=2) as midpool,
            tc.tile_pool(name="accpool", bufs=1) as accpool,
            tc.tile_pool(name="smallpool", bufs=1) as smallpool,
            tc.tile_pool(name="ps_t", bufs=4, space="PSUM") as ps_t,
            tc.tile_pool(name="ps_yw", bufs=2, space="PSUM") as ps_yw,
            tc.tile_pool(name="ps_b", bufs=2, space="PSUM") as ps_b,
        ):
            # Issue the first x cast-loads before anything else touches the
            # GpSimd queue so HBM streaming starts at the preamble's end.
            # Eighth-span grain (4 KiB contiguous f32 per partition, = one pb
            # iteration): the 16 SWDGE engines run 100% busy and their
            # per-byte rate IMPROVES with smaller rows (25.3 B/ns HBM at
            # 8 KiB rows vs 20.2 at 16 KiB), so use the finest grain that
            # matches the pb pipeline.
            ESF = SPAN_F // 8  # eighth-span: 32 t = 0.5 MiB of f32 in DRAM
            NL = NSPAN * 8
            PREF = 8           # lookahead in eighth-span loads (= 1 span)
            xq = []

            def issue_load(li):
                x_h = xpool.tile([128, ESF], BF16)
                nc.gpsimd.dma_start(
                    out=x_h[:], in_=x_ext[:, li * ESF:(li + 1) * ESF]
                )
                xq.append(x_h)

            issue_load(0)
            issue_load(1)

            ident = consts.tile([128, 128], F32)
            make_identity(nc, ident[:])
            ident_bf = consts.tile([128, 128], BF16)
            make_identity(nc, ident_bf[:])

            for li in range(2, PREF):
                issue_load(li)

            # Four stationary matrices A32_q [128, 32], q = 0..3.
            # Column m = 16*j + 4*q + t''; A32_q[(t', n), m] = delta(t', t'') *
            # (qb[n] if j == 0 else cs[n]); zero columns for other q values.
            # Stage qb/cs via the Activation HWDGE queue: the Sync queue
            # carries one companion descriptor per SWDGE load, so qbcs
            # staging there would delay the x stream.
            qbcs = consts.tile([128, 2], F32)
            for tp in range(4):
                nc.scalar.dma_start(out=qbcs[32 * tp:32 * tp + 32, 0:1], in_=qb_ext[:])
                nc.scalar.dma_start(out=qbcs[32 * tp:32 * tp + 32, 1:2], in_=cs_ext[:])
            A32 = []
            for q in range(4):
                Aq = consts.tile([128, 32], BF16, tag=f"A32_{q}")
                nc.vector.memset(Aq[:], 0.0)
                for tp in range(4):
                    sl = slice(32 * tp, 32 * tp + 32)
                    nc.vector.tensor_copy(
                        out=Aq[sl, 4 * q + tp:4 * q + tp + 1], in_=qbcs[sl, 0:1]
                    )
                    nc.vector.tensor_copy(
                        out=Aq[sl, 16 + 4 * q + tp:16 + 4 * q + tp + 1], in_=qbcs[sl, 1:2]
                    )
                A32.append(Aq)

            # Persistent per-core accumulators
            y_sb = accpool.tile([128, T], F32)
            w_sb = accpool.tile([128, T], F32)
            out_sb = accpool.tile([128, T], F32)

            m_term = smallpool.tile([128, C], F32)
            p_sc = smallpool.tile([128, C], F32)
            S_exc = smallpool.tile([128, C + 1], F32)
            pred = smallpool.tile([128, C + 1], F32)
            tau = smallpool.tile([128, C], F32)
            nc.vector.memset(S_exc[:, 0:1], 1.0)
            nc.vector.memset(pred[:, 0:1], 0.0)

            yw_mid_of = {}
            ps2_of = {}

            def emit_ywmid(sp, ps_y):
                yw_mid = midpool.tile([128, 512], F32)
                nc.scalar.copy(out=yw_mid[:], in_=ps_y[:])
                yw_mid_of[sp] = yw_mid

            def emit_backT(sp):
                yw_mid = yw_mid_of.pop(sp)
                ps2 = ps_b.tile([128, 512], F32)
                for blk2 in range(4):
                    nc.tensor.transpose(
                        ps2[:, blk2 * 128:(blk2 + 1) * 128],
                        yw_mid[:, blk2 * 128:(blk2 + 1) * 128],
                        ident[:],
                    )
                ps2_of[sp] = ps2

            def emit_ywcopies_reduce(sp):
                # ps2 free index = 128*blk + 32*g4 + 16*j + 4*q + t''
                # t(within span) = 64*g4 + 16*q + 4*blk + t''
                ps2 = ps2_of.pop(sp)
                ps2v = ps2[:].rearrange(
                    "p (b g j q t) -> p g q b j t", b=4, g=4, j=2, q=4, t=4
                )
                yspan = y_sb[:, sp * SPAN_T:(sp + 1) * SPAN_T].rearrange(
                    "p (g q b t) -> p g q b t", g=4, q=4, b=4, t=4
                )
                wspan = w_sb[:, sp * SPAN_T:(sp + 1) * SPAN_T].rearrange(
                    "p (g q b t) -> p g q b t", g=4, q=4, b=4, t=4
                )
                nc.vector.tensor_copy(out=yspan, in_=ps2v[:, :, :, :, 0:1, :].squeeze(4))
                nc.vector.tensor_copy(out=wspan, in_=ps2v[:, :, :, :, 1:2, :].squeeze(4))

                csl = slice(sp * CPS, (sp + 1) * CPS)
                nc.vector.tensor_reduce(
                    out=m_term[:, csl],
                    in_=y_sb[:, sp * SPAN_T:(sp + 1) * SPAN_T].rearrange(
                        "p (c s) -> p c s", c=CPS, s=ADAPT
                    ),
                    axis=mybir.AxisListType.X,
                    op=mybir.AluOpType.add,
                )
                nc.vector.tensor_reduce(
                    out=p_sc[:, csl],
                    in_=w_sb[:, sp * SPAN_T:(sp + 1) * SPAN_T].rearrange(
                        "p (c s) -> p c s", c=CPS, s=ADAPT
                    ),
                    axis=mybir.AxisListType.X,
                    op=mybir.AluOpType.add,
                )

            def emit_scans(sp):
                c_lo, c_hi = sp * CPS, (sp + 1) * CPS
                sl = slice(c_lo, c_hi)
                nc.vector.tensor_scalar_mul(m_term[:, sl], m_term[:, sl], 1.0 / ADAPT)
                nc.vector.tensor_scalar_mul(p_sc[:, sl], p_sc[:, sl], 1.0 / ADAPT)
                nc.vector.tensor_tensor_scan(
                    out=S_exc[:, c_lo + 1:c_hi + 1],
                    data0=p_sc[:, sl],
                    data1=p_sc[:, sl],
                    initial=S_exc[:, c_lo:c_lo + 1],
                    op0=mybir.AluOpType.mult,
                    op1=mybir.AluOpType.bypass,
                )
                nc.vector.tensor_mul(out=tau[:, sl], in0=S_exc[:, sl], in1=m_term[:, sl])
                nc.vector.tensor_tensor_scan(
                    out=pred[:, c_lo + 1:c_hi + 1],
                    data0=tau[:, sl],
                    data1=tau[:, sl],
                    initial=pred[:, c_lo:c_lo + 1],
                    op0=mybir.AluOpType.add,
                    op1=mybir.AluOpType.bypass,
                )

            def emit_affine_store(sp):
                c_lo, c_hi = sp * CPS, (sp + 1) * CPS
                sl = slice(c_lo, c_hi)
                t_lo, t_hi = c_lo * ADAPT, c_hi * ADAPT
                y3 = y_sb[:, t_lo:t_hi].rearrange("p (c s) -> p c s", c=CPS, s=ADAPT)
                o3 = out_sb[:, t_lo:t_hi].rearrange("p (c s) -> p c s", c=CPS, s=ADAPT)
                S_b = S_exc[:, sl].unsqueeze(2).broadcast_to([128, CPS, ADAPT])
                pred_b = pred[:, sl].unsqueeze(2).broadcast_to([128, CPS, ADAPT])
                nc.vector.tensor_mul(out=o3, in0=y3, in1=S_b)
                nc.vector.tensor_add(out=o3, in0=o3, in1=pred_b)
                nc.sync.dma_start(out=out_ext[:, t_lo:t_hi], in_=out_sb[:, t_lo:t_hi])

            for sp in range(NSPAN):
                xT_sp = xtpool.tile([128, SPAN_F], BF16)
                # y/w projection, interleaved with the transposes that feed it.
                # Group g = g4*4+q covers blocks 4g..4g+3 (t = 16g + 4blk + t'').
                # Strip g4 accumulates 4 matmuls into ps_y[32*g4 : 32*g4+32, :];
                # dense partition layout p = 32*g4 + 16*j + 4*q + t'', free=(blk,b).
                ps_y = ps_yw.tile([128, 512], F32)
                for pb in range(8):
                    # SWDGE cast-load f32 DRAM -> bf16 SBUF, eighth-span grain
                    li = sp * 8 + pb
                    if li + PREF < NL:
                        issue_load(li + PREF)
                    x_h = xq[li]
                    pst = ps_t.tile([128, 1024], BF16)
                    for k in range(8):
                        blk_l = k
                        nc.tensor.transpose(
                            pst[:, k * 128:(k + 1) * 128],
                            x_h[:, blk_l * 128:(blk_l + 1) * 128],
                            ident_bf[:],
                        )
                    dst = xT_sp[:, pb * 1024:(pb + 1) * 1024]
                    if pb % 2 == 0:
                        nc.vector.tensor_copy(out=dst, in_=pst[:])
                    else:
                        nc.scalar.copy(out=dst, in_=pst[:])
                    for gg in range(2):
                        g = pb * 2 + gg
                        g4, q = divmod(g, 4)
                        nc.tensor.matmul(
                            ps_y[32 * g4:32 * g4 + 32, :],
                            A32[q][:],
                            xT_sp[:, g * 512:(g + 1) * 512],
                            start=(q == 0),
                            stop=(q == 3),
                            tile_position=(0, 32 * g4),
                        )
                    # Previous span's post stages, placed where their deps are
                    # already met so they never head-of-line-block this span.
                    if sp > 0:
                        if pb == 1:
                            emit_backT(sp - 1)
                        elif pb == 3:
                            emit_ywcopies_reduce(sp - 1)
                        elif pb == 5:
                            emit_scans(sp - 1)
                        elif pb == 7:
                            emit_affine_store(sp - 1)

                emit_ywmid(sp, ps_y)

            last = NSPAN - 1
            emit_backT(last)
            emit_ywcopies_reduce(last)
            emit_scans(last)
            emit_affine_store(last)

    nc.compile()
    return nc


def kernel(x, quant_bins, change_scales):
    global _cached_nc
    if _cached_nc is None:
        _cached_nc = build_kernel()
    nc = _cached_nc

    x = np.ascontiguousarray(x, dtype=np.float32)
    qb = np.ascontiguousarray(quant_bins, dtype=np.float32).reshape(NB, 1)
    cs = np.ascontiguousarray(change_scales, dtype=np.float32).reshape(NB, 1)

    in_maps = [
        {
            "x": x[i * BS:(i + 1) * BS].reshape(BS, T * NB),
            "quant_bins": qb,
            "change_scales": cs,
        }
        for i in range(NCORES)
    ]
    res = run_bass_kernel_spmd(nc, in_maps, core_ids=list(range(NCORES)))
    out = np.concatenate([res.results[i]["out"] for i in range(NCORES)], axis=0)
    return out.astype(np.float32)


if __name__ == "__main__":
    rng = np.random.default_rng(0)
    x = rng.standard_normal((B, T, NB)).astype(np.float32)
    qb = rng.standard_normal((NB,)).astype(np.float32)
    cs = rng.uniform(0.9, 1.1, (NB, 1)).astype(np.float32)
    out = kernel(x=x, quant_bins=qb, change_scales=cs)
    print("out", out.shape, out.dtype)


# revision 16
# speedup vs baseline: 1.1960x; 1.0152x over previous
"""DeltaDequantization Trainium2 kernel (8-core SPMD, pure data parallel over batch).

Math (per batch element b, chunks c of 32 steps):
    scale_c = (1/32) * sum_{s,n} x[b,c,s,n] * cs[n]          (independent of carry!)
    S_c     = prod_{c'<c} scale_c'          (exclusive cumprod)
    y[b,t]  = sum_n x[b,t,n] * qb[n]
    m_c     = (1/32) * sum_{s in c} y[b,t]
    pred_c  = sum_{c'<c} S_c' * m_c'        (exclusive cumsum)
    out[b,t]= pred_c(t) + S_c(t) * y[b,t]

Kernel: load x naturally [128b, (t,n)], PE-transpose 128x128 blocks to get
(t',n) on partitions, one [128,8] matmul computes y and w=x@cs for 4 t's at a
time, PE-transpose back to [b,t], tensor_tensor_scan for the 64-step
recurrences, affine, store.

Pipelining notes (measured on HW):
- The 16 SWDGE DMA engines run 100% busy during the x stream; per-engine
  HBM-side rate peaks at ~25 B/ns with 8 KiB DRAM rows (quarter-span grain).
  Larger (16K) and smaller (4K) rows are slower.  The stream, not HBM, is
  the bottleneck, so total DMA bytes set an ~85 us floor for the window.
- Engine queues execute in (priority = program) order with head-of-line
  blocking, so span sp's post-matmul stages are EMITTED inside span sp+1's
  pb loop at points where their dependencies are already satisfied.
- The last span's post-processing runs at half-span granularity so only the
  final half-span's chain sits on the serial tail.
"""

import numpy as np
from contextlib import ExitStack

import concourse.bass as bass
import concourse.bacc as bacc
import concourse.tile as tile
from concourse import mybir
from concourse.bass_utils import run_bass_kernel_spmd
from concourse.masks import make_identity

F32 = mybir.dt.float32
BF16 = mybir.dt.bfloat16

B, T, NB = 1024, 2048, 32
NCORES = 8
BS = B // NCORES          # 128 batch rows per core = full partition dim
ADAPT = 32
C = T // ADAPT            # 64 chunks
SPAN_T = 256              # timesteps per pipelined span
NSPAN = T // SPAN_T       # 8
SPAN_F = SPAN_T * NB      # 8192 f32 elements per partition per span
CPS = SPAN_T // ADAPT     # 8 chunks per span

_cached_nc = None


def build_kernel():
    nc = bacc.Bacc("TRN2", target_bir_lowering=False, debug=False)

    x_ext = nc.dram_tensor("x", [BS, T * NB], F32, kind="ExternalInput")
    qb_ext = nc.dram_tensor("quant_bins", [NB, 1], F32, kind="ExternalInput")
    cs_ext = nc.dram_tensor("change_scales", [NB, 1], F32, kind="ExternalInput")
    out_ext = nc.dram_tensor("out", [BS, T], F32, kind="ExternalOutput")

    with tile.TileContext(nc) as tc:
        with (
            tc.tile_pool(name="consts", bufs=1) as consts,
            tc.tile_pool(name="xpool", bufs=10) as xpool,
            tc.tile_pool(name="xtpool", bufs=3) as xtpool,
            tc.tile_pool(name="midpool", bufs=2) as midpool,
            tc.tile_pool(name="accpool", bufs=1) as accpool,
            tc.tile_pool(name="smallpool", bufs=1) as smallpool,
            tc.tile_pool(name="ps_t", bufs=4, space="PSUM") as ps_t,
            tc.tile_pool(name="ps_yw", bufs=2, space="PSUM") as ps_yw,
            tc.tile_pool(name="ps_b", bufs=2, space="PSUM") as ps_b,
        ):
            # Issue the first x cast-loads before anything else touches the
            # GpSimd queue so HBM streaming starts at the preamble's end.
            QSF = SPAN_F // 4  # quarter-span: 64 t = 1 MiB of f32 in DRAM
            NQ = NSPAN * 4
            PREF = 8
            xq = []

            def issue_load(qi):
                x_h = xpool.tile([128, QSF], BF16)
                if qi == NQ - 1:
                    # Final quarter split in two for finer tail gating.
                    h = QSF // 2
                    nc.gpsimd.dma_start(
                        out=x_h[:, 0:h], in_=x_ext[:, qi * QSF:qi * QSF + h]
                    )
                    nc.gpsimd.dma_start(
                        out=x_h[:, h:QSF], in_=x_ext[:, qi * QSF + h:(qi + 1) * QSF]
                    )
                else:
                    nc.gpsimd.dma_start(
                        out=x_h[:], in_=x_ext[:, qi * QSF:(qi + 1) * QSF]
                    )
                xq.append(x_h)

            issue_load(0)
            issue_load(1)

            ident = consts.tile([128, 128], F32)
            make_identity(nc, ident[:])
            ident_bf = consts.tile([128, 128], BF16)
            make_identity(nc, ident_bf[:])

            for qi in range(2, PREF):
                issue_load(qi)

            # Four stationary matrices A32_q [128, 32], q = 0..3.
            # Column m = 16*j + 4*q + t''; A32_q[(t', n), m] = delta(t', t'') *
            # (qb[n] if j == 0 else cs[n]); zero columns for other q values.
            # Stage qb/cs via the Activation HWDGE queue: the Sync queue
            # carries one companion descriptor per SWDGE load.
            qbcs = consts.tile([128, 2], F32)
            for tp in range(4):
                nc.scalar.dma_start(out=qbcs[32 * tp:32 * tp + 32, 0:1], in_=qb_ext[:])
                nc.scalar.dma_start(out=qbcs[32 * tp:32 * tp + 32, 1:2], in_=cs_ext[:])
            A32 = []
            for q in range(4):
                Aq = consts.tile([128, 32], BF16, tag=f"A32_{q}")
                nc.vector.memset(Aq[:], 0.0)
                for tp in range(4):
                    sl = slice(32 * tp, 32 * tp + 32)
                    nc.vector.tensor_copy(
                        out=Aq[sl, 4 * q + tp:4 * q + tp + 1], in_=qbcs[sl, 0:1]
                    )
                    nc.vector.tensor_copy(
                        out=Aq[sl, 16 + 4 * q + tp:16 + 4 * q + tp + 1], in_=qbcs[sl, 1:2]
                    )
                A32.append(Aq)

            # Persistent per-core accumulators
            y_sb = accpool.tile([128, T], F32)
            w_sb = accpool.tile([128, T], F32)
            out_sb = accpool.tile([128, T], F32)

            m_term = smallpool.tile([128, C], F32)
            p_sc = smallpool.tile([128, C], F32)
            S_exc = smallpool.tile([128, C + 1], F32)
            pred = smallpool.tile([128, C + 1], F32)
            tau = smallpool.tile([128, C], F32)
            nc.vector.memset(S_exc[:, 0:1], 1.0)
            nc.vector.memset(pred[:, 0:1], 0.0)

            yw_mid_of = {}
            ps2_of = {}

            def emit_ywmid(sp, ps_y):
                yw_mid = midpool.tile([128, 512], F32)
                nc.scalar.copy(out=yw_mid[:], in_=ps_y[:])
                yw_mid_of[sp] = yw_mid

            def emit_backT(sp):
                yw_mid = yw_mid_of.pop(sp)
                ps2 = ps_b.tile([128, 512], F32)
                for blk2 in range(4):
                    nc.tensor.transpose(
                        ps2[:, blk2 * 128:(blk2 + 1) * 128],
                        yw_mid[:, blk2 * 128:(blk2 + 1) * 128],
                        ident[:],
                    )
                ps2_of[sp] = ps2

            def emit_ywcopies_reduce(sp):
                # ps2 free index = 128*blk + 32*g4 + 16*j + 4*q + t''
                # t(within span) = 64*g4 + 16*q + 4*blk + t''
                ps2 = ps2_of.pop(sp)
                ps2v = ps2[:].rearrange(
                    "p (b g j q t) -> p g q b j t", b=4, g=4, j=2, q=4, t=4
                )
                yspan = y_sb[:, sp * SPAN_T:(sp + 1) * SPAN_T].rearrange(
                    "p (g q b t) -> p g q b t", g=4, q=4, b=4, t=4
                )
                wspan = w_sb[:, sp * SPAN_T:(sp + 1) * SPAN_T].rearrange(
                    "p (g q b t) -> p g q b t", g=4, q=4, b=4, t=4
                )
                nc.vector.tensor_copy(out=yspan, in_=ps2v[:, :, :, :, 0:1, :].squeeze(4))
                nc.vector.tensor_copy(out=wspan, in_=ps2v[:, :, :, :, 1:2, :].squeeze(4))

                csl = slice(sp * CPS, (sp + 1) * CPS)
                nc.vector.tensor_reduce(
                    out=m_term[:, csl],
                    in_=y_sb[:, sp * SPAN_T:(sp + 1) * SPAN_T].rearrange(
                        "p (c s) -> p c s", c=CPS, s=ADAPT
                    ),
                    axis=mybir.AxisListType.X,
                    op=mybir.AluOpType.add,
                )
                nc.vector.tensor_reduce(
                    out=p_sc[:, csl],
                    in_=w_sb[:, sp * SPAN_T:(sp + 1) * SPAN_T].rearrange(
                        "p (c s) -> p c s", c=CPS, s=ADAPT
                    ),
                    axis=mybir.AxisListType.X,
                    op=mybir.AluOpType.add,
                )

            def emit_scans(c_lo, c_hi):
                sl = slice(c_lo, c_hi)
                nc.vector.tensor_scalar_mul(m_term[:, sl], m_term[:, sl], 1.0 / ADAPT)
                nc.vector.tensor_scalar_mul(p_sc[:, sl], p_sc[:, sl], 1.0 / ADAPT)
                nc.vector.tensor_tensor_scan(
                    out=S_exc[:, c_lo + 1:c_hi + 1],
                    data0=p_sc[:, sl],
                    data1=p_sc[:, sl],
                    initial=S_exc[:, c_lo:c_lo + 1],
                    op0=mybir.AluOpType.mult,
                    op1=mybir.AluOpType.bypass,
                )
                nc.vector.tensor_mul(out=tau[:, sl], in0=S_exc[:, sl], in1=m_term[:, sl])
                nc.vector.tensor_tensor_scan(
                    out=pred[:, c_lo + 1:c_hi + 1],
                    data0=tau[:, sl],
                    data1=tau[:, sl],
                    initial=pred[:, c_lo:c_lo + 1],
                    op0=mybir.AluOpType.add,
                    op1=mybir.AluOpType.bypass,
                )

            def emit_affine_store(c_lo, c_hi):
                sl = slice(c_lo, c_hi)
                nch = c_hi - c_lo
                t_lo, t_hi = c_lo * ADAPT, c_hi * ADAPT
                y3 = y_sb[:, t_lo:t_hi].rearrange("p (c s) -> p c s", c=nch, s=ADAPT)
                o3 = out_sb[:, t_lo:t_hi].rearrange("p (c s) -> p c s", c=nch, s=ADAPT)
                S_b = S_exc[:, sl].unsqueeze(2).broadcast_to([128, nch, ADAPT])
                pred_b = pred[:, sl].unsqueeze(2).broadcast_to([128, nch, ADAPT])
                nc.vector.tensor_mul(out=o3, in0=y3, in1=S_b)
                nc.vector.tensor_add(out=o3, in0=o3, in1=pred_b)
                nc.sync.dma_start(out=out_ext[:, t_lo:t_hi], in_=out_sb[:, t_lo:t_hi])

            # ---- half-span post stages (last span only) ----
            ps2h_of = {}

            def emit_ywmid_h(sp, h, ps_y):
                yw_mid = midpool.tile([128, 512], F32)
                nc.scalar.copy(out=yw_mid[64 * h:64 * h + 64, :],
                               in_=ps_y[64 * h:64 * h + 64, :])
                yw_mid_of[(sp, h)] = yw_mid

            def emit_backT_h(sp, h):
                yw_mid = yw_mid_of.pop((sp, h))
                ps2 = ps_b.tile([128, 256], F32)
                for blk2 in range(4):
                    nc.tensor.transpose(
                        ps2[:, blk2 * 64:(blk2 + 1) * 64],
                        yw_mid[64 * h:64 * h + 64, blk2 * 128:(blk2 + 1) * 128],
                        ident[64 * h:64 * h + 64, 64 * h:64 * h + 64],
                    )
                ps2h_of[(sp, h)] = ps2

            def emit_ywcopies_reduce_h(sp, h):
                # ps2 free index = 64*blk + 32*gl + 16*j + 4*q + t''  (gl = g4-2h)
                # t(within span) = 128*h + 64*gl + 16*q + 4*blk + t''
                ps2 = ps2h_of.pop((sp, h))
                ps2v = ps2[:].rearrange(
                    "p (b g j q t) -> p g q b j t", b=4, g=2, j=2, q=4, t=4
                )
                t0 = sp * SPAN_T + 128 * h
                yspan = y_sb[:, t0:t0 + 128].rearrange(
                    "p (g q b t) -> p g q b t", g=2, q=4, b=4, t=4
                )
                wspan = w_sb[:, t0:t0 + 128].rearrange(
                    "p (g q b t) -> p g q b t", g=2, q=4, b=4, t=4
                )
                nc.vector.tensor_copy(out=yspan, in_=ps2v[:, :, :, :, 0:1, :].squeeze(4))
                nc.vector.tensor_copy(out=wspan, in_=ps2v[:, :, :, :, 1:2, :].squeeze(4))

                c0 = sp * CPS + 4 * h
                csl = slice(c0, c0 + 4)
                nc.vector.tensor_reduce(
                    out=m_term[:, csl],
                    in_=y_sb[:, t0:t0 + 128].rearrange(
                        "p (c s) -> p c s", c=4, s=ADAPT
                    ),
                    axis=mybir.AxisListType.X,
                    op=mybir.AluOpType.add,
                )
                nc.vector.tensor_reduce(
                    out=p_sc[:, csl],
                    in_=w_sb[:, t0:t0 + 128].rearrange(
                        "p (c s) -> p c s", c=4, s=ADAPT
                    ),
                    axis=mybir.AxisListType.X,
                    op=mybir.AluOpType.add,
                )

            LAST = NSPAN - 1
            for sp in range(NSPAN):
                xT_sp = xtpool.tile([128, SPAN_F], BF16)
                # y/w projection, interleaved with the transposes that feed it.
                # Group g = g4*4+q covers blocks 4g..4g+3 (t = 16g + 4blk + t'').
                # Strip g4 accumulates 4 matmuls into ps_y[32*g4 : 32*g4+32, :];
                # dense partition layout p = 32*g4 + 16*j + 4*q + t'', free=(blk,b).
                ps_y = ps_yw.tile([128, 512], F32)
                for pb in range(8):
                    # SWDGE cast-load f32 DRAM -> bf16 SBUF, quarter-span grain
                    if pb % 2 == 0:
                        qi = sp * 4 + pb // 2
                        if qi + PREF < NQ:
                            issue_load(qi + PREF)
                        x_h = xq[qi]
                    pst = ps_t.tile([128, 1024], BF16)
                    for k in range(8):
                        blk_l = (pb % 2) * 8 + k
                        nc.tensor.transpose(
                            pst[:, k * 128:(k + 1) * 128],
                            x_h[:, blk_l * 128:(blk_l + 1) * 128],
                            ident_bf[:],
                        )
                    dst = xT_sp[:, pb * 1024:(pb + 1) * 1024]
                    if pb % 2 == 0:
                        nc.vector.tensor_copy(out=dst, in_=pst[:])
                    else:
                        nc.scalar.copy(out=dst, in_=pst[:])
                    for gg in range(2):
                        g = pb * 2 + gg
                        g4, q = divmod(g, 4)
                        nc.tensor.matmul(
                            ps_y[32 * g4:32 * g4 + 32, :],
                            A32[q][:],
                            xT_sp[:, g * 512:(g + 1) * 512],
                            start=(q == 0),
                            stop=(q == 3),
                            tile_position=(0, 32 * g4),
                        )
                    # Previous span's post stages, placed where their deps are
                    # already met so they never head-of-line-block this span.
                    if sp > 0:
                        if pb == 1:
                            emit_backT(sp - 1)
                        elif pb == 2:
                            emit_ywcopies_reduce(sp - 1)
                        elif pb == 3:
                            emit_scans((sp - 1) * CPS, sp * CPS)
                        elif pb == 4:
                            emit_affine_store((sp - 1) * CPS, sp * CPS)
                    # Last span: first half's post stages start as soon as
                    # strips 0-1 finish (after pb3's matmuls).
                    if sp == LAST:
                        if pb == 5:
                            emit_ywmid_h(sp, 0, ps_y)
                        elif pb == 6:
                            emit_backT_h(sp, 0)
                        elif pb == 7:
                            emit_ywcopies_reduce_h(sp, 0)

                if sp < LAST:
                    emit_ywmid(sp, ps_y)
                else:
                    c0 = sp * CPS
                    emit_scans(c0, c0 + 4)
                    emit_affine_store(c0, c0 + 4)
                    emit_ywmid_h(sp, 1, ps_y)
                    emit_backT_h(sp, 1)
                    emit_ywcopies_reduce_h(sp, 1)
                    emit_scans(c0 + 4, c0 + 8)
                    emit_affine_store(c0 + 4, c0 + 8)

    nc.compile()
    return nc


def kernel(x, quant_bins, change_scales):
    global _cached_nc
    if _cached_nc is None:
        _cached_nc = build_kernel()
    nc = _cached_nc

    x = np.ascontiguousarray(x, dtype=np.float32)
    qb = np.ascontiguousarray(quant_bins, dtype=np.float32).reshape(NB, 1)
    cs = np.ascontiguousarray(change_scales, dtype=np.float32).reshape(NB, 1)

    in_maps = [
        {
            "x": x[i * BS:(i + 1) * BS].reshape(BS, T * NB),
            "quant_bins": qb,
            "change_scales": cs,
        }
        for i in range(NCORES)
    ]
    res = run_bass_kernel_spmd(nc, in_maps, core_ids=list(range(NCORES)))
    out = np.concatenate([res.results[i]["out"] for i in range(NCORES)], axis=0)
    return out.astype(np.float32)


if __name__ == "__main__":
    rng = np.random.default_rng(0)
    x = rng.standard_normal((B, T, NB)).astype(np.float32)
    qb = rng.standard_normal((NB,)).astype(np.float32)
    cs = rng.uniform(0.9, 1.1, (NB, 1)).astype(np.float32)
    out = kernel(x=x, quant_bins=qb, change_scales=cs)
    print("out", out.shape, out.dtype)
